# revision 65
# baseline (speedup 1.0000x reference)
"""Trainium2 Bass kernel for nn_NetworkStitch_5016521802529.

Cost-volume block: f1n = l2_normalize_c(feature1); hori/verti 9-offset
correlation bands vs feature2; leaky_relu; concat -> [B, 18, H, W].

Strategy (pure data-parallel over batch, 32 samples/core on 8 cores):
  - SWDGE casting DMA loads f32 DRAM -> bf16 SBUF, 4 samples per
    [128, 2, H, W] tile (sample half on partition halves, pair slot on
    dim 1) -- 4 MB reads amortize the ~2us SWDGE fixed cost.
  - s^2 = sum_c f1^2 via DVE square + ones-stationary matmul (output
    replicated across partitions); single ACT Abs_reciprocal_sqrt
    (scale=64^2) gives s_inv = 1/(64*s) in the same layout.
  - f1n = f1b * s_inv (bf16).
  - Per h (hori) / per w (verti): single-line Gram G = f1n_l^T @ f2b_l
    ([64,64], K=64) on 64x64 PE quadrants; the two samples' matmuls are
    interleaved so consecutive LDWEIGHTS hit different row groups (PE
    pulls them ahead of in-flight MATMULs). 16 Grams per PSUM bank
    [128, 512] (partition half = line%2, free slot = (line%16)//2,
    bank = line//16).
  - Eviction PSUM->SBUF fused with leaky-relu: banks 0-1 via ACT Lrelu
    (alpha=.01), banks 2-3 via DVE copy + scalar_tensor_tensor --
    balances the two engines.
  - Per sample one [128, 2, 2048] bf16 strip (both directions) stored
    with a single 1 MB DMA; host extracts the 9-diagonal bands (pure
    indexing) during unshard.
"""

import os
import sys

sys.path.insert(0, "/opt/trn_rl_repo")

import numpy as np
import ml_dtypes

import concourse.bacc as bacc
import concourse.bass as bass
import concourse.tile as tile
from concourse import mybir

B, C, H, W = 256, 64, 64, 64
N_CORES = 8
B_CORE = B // N_CORES
SR = 4
ND = 2 * SR + 1  # 9
HW = H * W
BF16 = mybir.dt.bfloat16
F32 = mybir.dt.float32

# Eviction split: banks < this evicted by ACT(Lrelu), rest by DVE
# (copy + stt; stt cannot read two PSUM operands).
ACT_EVICT_BANKS = 2


def build_nc(bcore=B_CORE, act_evict_banks=ACT_EVICT_BANKS):
    """Build the per-core Bass graph for `bcore` samples (mult of 4)."""
    assert bcore % 4 == 0
    nc = bacc.Bacc("TRN2", target_bir_lowering=False, debug=False)
    # Inputs staged as bf16 by the host (bit-identical to the bf16 cast
    # the kernel would do on load anyway) -- halves input HBM traffic
    # and lets the loads use HWDGE instead of casting SWDGE. The host
    # also pre-permutes them into the exact SBUF tile layout
    # [group, partition = (s%2)*64 + c, pair slot = (s//2)%2, h, w] so
    # each load is one fully contiguous descriptor pattern.
    f1d = nc.dram_tensor(
        "f1", [bcore // 4, 128, 2, H, W], BF16, kind="ExternalInput"
    )
    f2d = nc.dram_tensor(
        "f2", [bcore // 4, 128, 2, H, W], BF16, kind="ExternalInput"
    )
    # per pair: [128 partitions, 2 samples, 2 directions, 2048] bf16 strips
    outd = nc.dram_tensor(
        "out", [bcore // 2, 128, 2, 2, 4 * 512], BF16, kind="ExternalOutput"
    )
    ngrp = bcore // 4

    from contextlib import ExitStack

    with tile.TileContext(nc) as tc, ExitStack() as ctx:
        io = ctx.enter_context(tc.tile_pool(name="io", bufs=2))
        work = ctx.enter_context(tc.tile_pool(name="work", bufs=2))
        strips = ctx.enter_context(tc.tile_pool(name="strips", bufs=3))
        const = ctx.enter_context(tc.tile_pool(name="const", bufs=1))
        gram = ctx.enter_context(tc.tile_pool(name="gram", bufs=3, space="PSUM"))
        nrm = ctx.enter_context(tc.tile_pool(name="nrm", bufs=2, space="PSUM"))

        ones_t = const.tile([128, 64], BF16)
        nc.vector.memset(ones_t[:], 1.0)
        alpha01 = const.tile([128, 1], F32)
        nc.vector.memset(alpha01[:], 0.01)

        for grp in range(ngrp):
            # ---- loads: 4 samples -> [128, 2, H, W] bf16 ----
            if grp == 0:
                # cold start: per-pair tiles with their own 1 MB loads so
                # pair-0 compute begins as soon as its slice lands (deps
                # are tile-granular)
                f1v, f2v = [], []
                for j in range(2):
                    f1p = io.tile([128, H, W], BF16, name=f"f1p{j}",
                                  tag=f"f1p{j}", bufs=1)
                    f2p = io.tile([128, H, W], BF16, name=f"f2p{j}",
                                  tag=f"f2p{j}", bufs=1)
                    nc.sync.dma_start(out=f1p[:], in_=f1d[0][:, j])
                    nc.sync.dma_start(out=f2p[:], in_=f2d[0][:, j])
                    f1v.append(f1p[:])
                    f2v.append(f2p[:])
            else:
                f1b = io.tile([128, 2, H, W], BF16, tag="f1b")
                f2b = io.tile([128, 2, H, W], BF16, tag="f2b")
                nc.sync.dma_start(out=f1b[:], in_=f1d[grp])
                nc.sync.dma_start(out=f2b[:], in_=f2d[grp])
                f1v = [f1b[:, 0], f1b[:, 1]]
                f2v = [f2b[:, 0], f2b[:, 1]]

            # ---- norms for both pairs first (batches the ACT rsqrt ops
            # so the activation table only switches twice per group) ----
            f1ns = []
            for j in range(2):  # pair slot within the 4-sample group
                f1sq = work.tile([128, HW], BF16, name=f"f1sq{j}",
                                 tag="f1sq")
                nc.vector.tensor_mul(
                    f1sq[:],
                    f1v[j].rearrange("p h w -> p (h w)"),
                    f1v[j].rearrange("p h w -> p (h w)"),
                )
                s_inv = work.tile([128, HW], BF16, name=f"sinv{j}",
                                  tag="sinv")
                for ch in range(8):
                    ps = nrm.tile([128, 512], F32, tag="nps")
                    sl = slice(ch * 512, (ch + 1) * 512)
                    for half in range(2):
                        po = half * 64
                        nc.tensor.matmul(
                            out=ps[po : po + 64, :],
                            lhsT=ones_t[po : po + 64, :],
                            rhs=f1sq[po : po + 64, sl],
                            tile_position=(po, po),
                        )
                    # 1/sqrt(64^2 * ps) = 1/(64*s) in one ACT op (sum >=
                    # 0 so the |x| in Abs_reciprocal_sqrt is a no-op).
                    nc.scalar.activation(
                        out=s_inv[:, sl], in_=ps[:],
                        func=mybir.ActivationFunctionType.Abs_reciprocal_sqrt,
                        scale=float(64 * 64),
                    )

                f1n = work.tile([128, H, W], BF16, name=f"f1n{j}",
                                tag="f1n")
                nc.vector.tensor_mul(
                    f1n[:].rearrange("p h w -> p (h w)"),
                    f1v[j].rearrange("p h w -> p (h w)"),
                    s_inv[:],
                )
                f1ns.append(f1n)

            for j in range(2):
                f1n = f1ns[j]
                # ---- Grams (samples interleaved for LDW overlap) ----
                st = strips.tile([128, 2, 2, 4 * 512], BF16, tag="strip")
                for direction in range(2):  # 0 = hori (per h), 1 = verti
                    for g in range(4):  # bank group of 16 lines
                        # both samples' banks in one contiguous 2-bank
                        # tile so each eviction is a single [128, 1024]
                        # op (halves per-op overhead on the eviction
                        # engines, which pace the steady state)
                        psb = gram.tile([128, 2 * 512], F32, tag="gps")
                        ps2 = [psb[:, 0:512], psb[:, 512:1024]]
                        for i in range(16):
                            line = g * 16 + i
                            half, slot = i % 2, i // 2
                            for smp in range(2):
                                ko = smp * 64
                                if direction == 0:
                                    lhsT = f1n[ko : ko + 64, line, :]
                                    rhs = f2v[j][ko : ko + 64, line, :]
                                else:
                                    lhsT = f1n[ko : ko + 64, :, line]
                                    rhs = f2v[j][ko : ko + 64, :, line]
                                nc.tensor.matmul(
                                    out=ps2[smp][
                                        half * 64 : half * 64 + 64,
                                        slot * 64 : slot * 64 + 64,
                                    ],
                                    lhsT=lhsT,
                                    rhs=rhs,
                                    tile_position=(ko, half * 64),
                                )
                        osl = slice(g * 512, (g + 1) * 512)
                        # evict both samples in one op: dst is a 3D AP
                        # over the per-sample strip regions
                        dst = st[:, :, direction, osl]
                        src_ = psb[:].rearrange("p (s c) -> p s c", s=2)
                        if g < act_evict_banks or (
                            g == 3 and direction == 0
                        ):
                            # ACT fused evict + leaky from PSUM
                            nc.scalar.activation(
                                out=dst, in_=src_,
                                func=mybir.ActivationFunctionType.Lrelu,
                                alpha=alpha01[:],
                            )
                        else:
                            # DVE evict copy PSUM->SBUF bf16 (leaky
                            # applied in the batched stt below)
                            nc.vector.tensor_copy(out=dst, in_=src_)
                    # one batched leaky pass over the copy-evicted span
                    # (stt cannot take two PSUM reads); g3/dir0 was
                    # ACT-evicted so that span stops at bank g3
                    dlo = act_evict_banks * 512
                    dhi = 1536 if direction == 0 else 2048
                    dvs = st[:, :, direction, dlo:dhi]
                    nc.vector.scalar_tensor_tensor(
                        out=dvs,
                        in0=dvs,
                        scalar=0.01,
                        in1=dvs,
                        op0=mybir.AluOpType.mult,
                        op1=mybir.AluOpType.max,
                    )
                if grp == ngrp - 1 and j == 1:
                    # fan the tail store across queues so the final
                    # drain is short
                    for smp in range(2):
                        for direction in range(2):
                            nc.sync.dma_start(
                                out=outd[2 * grp + j, :, smp, direction],
                                in_=st[:, smp, direction],
                            )
                else:
                    nc.sync.dma_start(out=outd[2 * grp + j], in_=st[:])

    nc.compile()
    return nc


_NC_CACHE = {}


def _get_nc(bcore=B_CORE):
    if bcore not in _NC_CACHE:
        _NC_CACHE[bcore] = build_nc(bcore)
    return _NC_CACHE[bcore]


def _extract_bands(strips):
    """strips: [bcore//2, 128, 2, 2, 2048] float32-ish -> [bcore, 18, H, W].

    Gram line L (h for hori, w for verti) of sample 2*pr+smp: G_L[r, c] =
      strips[pr, (L%2)*64 + r, smp, dir, (L//16)*512 + ((L%16)//2)*64 + c].
    hori[d, h, w] = G_h[w, w+d-4]; verti[d, h, w] = Gv_w[h, h+d-4].
    """
    bcore = strips.shape[0] * 2
    s = np.asarray(strips, dtype=np.float32)
    # [pr, half(2), r(64), smp(2), dir(2), bank(4), slot(8), c(64)]
    s = s.reshape(bcore // 2, 2, 64, 2, 2, 4, 8, 64)
    # line index L = bank*16 + slot*2 + half -> G[(pr, smp), dir, L, r, c]
    g = s.transpose(0, 3, 4, 5, 6, 1, 2, 7).reshape(bcore, 2, 64, 64, 64)
    out = np.zeros((bcore, 2, ND, 64, 64), dtype=np.float32)
    idx = np.arange(64)
    for d in range(ND):
        o = d - SR
        lo, hi = max(0, -o), min(64, 64 - o)
        r = idx[lo:hi]
        # advanced idxs (incl. the int) are slice-separated -> dims lead:
        # result [len(r), b, L]
        hvals = g[:, 0, :, r, r + o]  # [w-valid, b, h=L]
        vvals = g[:, 1, :, r, r + o]  # [h-valid, b, w=L]
        out[:, 0, d, :, lo:hi] = hvals.transpose(1, 2, 0)
        out[:, 1, d, lo:hi, :] = vvals.transpose(1, 0, 2)
    return out.reshape(bcore, 2 * ND, 64, 64)


def _stage(x):
    """[B, C, H, W] f32 -> bf16 in [B//4, (s%2)*64+c, (s//2)%2, H, W]."""
    b = x.shape[0]
    xb = np.asarray(x, dtype=np.float32).astype(ml_dtypes.bfloat16)
    # sample s = 4*grp + 2*j + half -> [grp, j, half, c, h, w]
    xb = xb.reshape(b // 4, 2, 2, C, H, W)
    # -> [grp, (half, c), j, h, w]
    return np.ascontiguousarray(xb.transpose(0, 2, 3, 1, 4, 5)).reshape(
        b // 4, 128, 2, H, W
    )


def kernel(feature1, feature2, search_range):
    assert int(search_range) == SR
    f1 = _stage(feature1)
    f2 = _stage(feature2)
    bcore = f1.shape[0] * 4 // N_CORES
    nc = _get_nc(bcore)

    from concourse.bass_utils import run_bass_kernel_spmd

    gcore = bcore // 4
    in_maps = [
        {
            "f1": f1[c * gcore : (c + 1) * gcore],
            "f2": f2[c * gcore : (c + 1) * gcore],
        }
        for c in range(N_CORES)
    ]
    res = run_bass_kernel_spmd(nc, in_maps, list(range(N_CORES)))
    outs = [
        _extract_bands(res.results[c]["out"].astype(np.float32))
        for c in range(N_CORES)
    ]
    return np.concatenate(outs, axis=0)


# revision 66
# speedup vs baseline: 1.0326x; 1.0326x over previous
"""Trainium2 Bass kernel for nn_NetworkStitch_5016521802529.

Cost-volume block: f1n = l2_normalize_c(feature1); hori/verti 9-offset
correlation bands vs feature2; leaky_relu; concat -> [B, 18, H, W].

Strategy (pure data-parallel over batch, 32 samples/core on 8 cores):
  - SWDGE casting DMA loads f32 DRAM -> bf16 SBUF, 4 samples per
    [128, 2, H, W] tile (sample half on partition halves, pair slot on
    dim 1) -- 4 MB reads amortize the ~2us SWDGE fixed cost.
  - s^2 = sum_c f1^2 via DVE square + ones-stationary matmul (output
    replicated across partitions); single ACT Abs_reciprocal_sqrt
    (scale=64^2) gives s_inv = 1/(64*s) in the same layout.
  - f1n = f1b * s_inv (bf16).
  - Per h (hori) / per w (verti): single-line Gram G = f1n_l^T @ f2b_l
    ([64,64], K=64) on 64x64 PE quadrants; the two samples' matmuls are
    interleaved so consecutive LDWEIGHTS hit different row groups (PE
    pulls them ahead of in-flight MATMULs). 16 Grams per PSUM bank
    [128, 512] (partition half = line%2, free slot = (line%16)//2,
    bank = line//16).
  - Eviction PSUM->SBUF fused with leaky-relu: banks 0-1 via ACT Lrelu
    (alpha=.01), banks 2-3 via DVE copy + scalar_tensor_tensor --
    balances the two engines.
  - Per sample one [128, 2, 2048] bf16 strip (both directions) stored
    with a single 1 MB DMA; host extracts the 9-diagonal bands (pure
    indexing) during unshard.
"""

import os
import sys

sys.path.insert(0, "/opt/trn_rl_repo")

import numpy as np
import ml_dtypes

import concourse.bacc as bacc
import concourse.bass as bass
import concourse.tile as tile
from concourse import mybir

B, C, H, W = 256, 64, 64, 64
N_CORES = 8
B_CORE = B // N_CORES
SR = 4
ND = 2 * SR + 1  # 9
HW = H * W
BF16 = mybir.dt.bfloat16
F32 = mybir.dt.float32

# Eviction split: banks < this evicted by ACT(Lrelu), rest by DVE
# (copy + stt; stt cannot read two PSUM operands).
ACT_EVICT_BANKS = 2


def build_nc(bcore=B_CORE, act_evict_banks=ACT_EVICT_BANKS):
    """Build the per-core Bass graph for `bcore` samples (mult of 4)."""
    assert bcore % 4 == 0
    nc = bacc.Bacc("TRN2", target_bir_lowering=False, debug=False)
    # Inputs staged as bf16 by the host (bit-identical to the bf16 cast
    # the kernel would do on load anyway) -- halves input HBM traffic
    # and lets the loads use HWDGE instead of casting SWDGE. The host
    # also pre-permutes them into the exact SBUF tile layout
    # [group, partition = (s%2)*64 + c, pair slot = (s//2)%2, h, w] so
    # each load is one fully contiguous descriptor pattern.
    f1d = nc.dram_tensor(
        "f1", [bcore // 4, 128, 2, H, W], BF16, kind="ExternalInput"
    )
    f2d = nc.dram_tensor(
        "f2", [bcore // 4, 128, 2, H, W], BF16, kind="ExternalInput"
    )
    # per pair: [128 partitions, 2 samples, 2 directions, 2048] bf16 strips
    outd = nc.dram_tensor(
        "out", [bcore // 2, 128, 2, 2, 4 * 512], BF16, kind="ExternalOutput"
    )
    ngrp = bcore // 4

    from contextlib import ExitStack

    with tile.TileContext(nc) as tc, ExitStack() as ctx:
        io = ctx.enter_context(tc.tile_pool(name="io", bufs=2))
        work = ctx.enter_context(tc.tile_pool(name="work", bufs=2))
        strips = ctx.enter_context(tc.tile_pool(name="strips", bufs=3))
        const = ctx.enter_context(tc.tile_pool(name="const", bufs=1))
        gram = ctx.enter_context(tc.tile_pool(name="gram", bufs=6, space="PSUM"))
        nrm = ctx.enter_context(tc.tile_pool(name="nrm", bufs=2, space="PSUM"))

        ones_t = const.tile([128, 64], BF16)
        nc.vector.memset(ones_t[:], 1.0)
        alpha01 = const.tile([128, 1], F32)
        nc.vector.memset(alpha01[:], 0.01)

        for grp in range(ngrp):
            # ---- loads: 4 samples -> [128, 2, H, W] bf16 ----
            if grp == 0:
                # cold start: per-pair tiles with their own 1 MB loads so
                # pair-0 compute begins as soon as its slice lands (deps
                # are tile-granular)
                f1v, f2v = [], []
                for j in range(2):
                    f1p = io.tile([128, H, W], BF16, name=f"f1p{j}",
                                  tag=f"f1p{j}", bufs=1)
                    f2p = io.tile([128, H, W], BF16, name=f"f2p{j}",
                                  tag=f"f2p{j}", bufs=1)
                    nc.sync.dma_start(out=f1p[:], in_=f1d[0][:, j])
                    nc.sync.dma_start(out=f2p[:], in_=f2d[0][:, j])
                    f1v.append(f1p[:])
                    f2v.append(f2p[:])
            else:
                f1b = io.tile([128, 2, H, W], BF16, tag="f1b")
                f2b = io.tile([128, 2, H, W], BF16, tag="f2b")
                nc.sync.dma_start(out=f1b[:], in_=f1d[grp])
                nc.sync.dma_start(out=f2b[:], in_=f2d[grp])
                f1v = [f1b[:, 0], f1b[:, 1]]
                f2v = [f2b[:, 0], f2b[:, 1]]

            # ---- norms for both pairs first (batches the ACT rsqrt ops
            # so the activation table only switches twice per group) ----
            f1ns = []
            for j in range(2):  # pair slot within the 4-sample group
                f1sq = work.tile([128, HW], BF16, name=f"f1sq{j}",
                                 tag="f1sq")
                nc.vector.tensor_mul(
                    f1sq[:],
                    f1v[j].rearrange("p h w -> p (h w)"),
                    f1v[j].rearrange("p h w -> p (h w)"),
                )
                s_inv = work.tile([128, HW], BF16, name=f"sinv{j}",
                                  tag="sinv")
                for ch in range(8):
                    ps = nrm.tile([128, 512], F32, tag="nps")
                    sl = slice(ch * 512, (ch + 1) * 512)
                    for half in range(2):
                        po = half * 64
                        nc.tensor.matmul(
                            out=ps[po : po + 64, :],
                            lhsT=ones_t[po : po + 64, :],
                            rhs=f1sq[po : po + 64, sl],
                            tile_position=(po, po),
                        )
                    # 1/sqrt(64^2 * ps) = 1/(64*s) in one ACT op (sum >=
                    # 0 so the |x| in Abs_reciprocal_sqrt is a no-op).
                    nc.scalar.activation(
                        out=s_inv[:, sl], in_=ps[:],
                        func=mybir.ActivationFunctionType.Abs_reciprocal_sqrt,
                        scale=float(64 * 64),
                    )

                f1n = work.tile([128, H, W], BF16, name=f"f1n{j}",
                                tag="f1n")
                nc.vector.tensor_mul(
                    f1n[:].rearrange("p h w -> p (h w)"),
                    f1v[j].rearrange("p h w -> p (h w)"),
                    s_inv[:],
                )
                f1ns.append(f1n)

            for j in range(2):
                f1n = f1ns[j]
                # ---- Grams (samples interleaved for LDW overlap) ----
                st = strips.tile([128, 2, 2, 4 * 512], BF16, tag="strip")
                for direction in range(2):  # 0 = hori (per h), 1 = verti
                    for g in range(4):  # bank group of 16 lines
                        # one tag, two calls: each bank is its own pool
                        # generation so PSUM recycles per-bank, not
                        # per-bank-pair
                        ps2 = [
                            gram.tile([128, 512], F32,
                                      name=f"gps{s}", tag="gps")
                            for s in range(2)
                        ]
                        for i in range(16):
                            line = g * 16 + i
                            half, slot = i % 2, i // 2
                            for smp in range(2):
                                ko = smp * 64
                                if direction == 0:
                                    lhsT = f1n[ko : ko + 64, line, :]
                                    rhs = f2v[j][ko : ko + 64, line, :]
                                else:
                                    lhsT = f1n[ko : ko + 64, :, line]
                                    rhs = f2v[j][ko : ko + 64, :, line]
                                nc.tensor.matmul(
                                    out=ps2[smp][
                                        half * 64 : half * 64 + 64,
                                        slot * 64 : slot * 64 + 64,
                                    ],
                                    lhsT=lhsT,
                                    rhs=rhs,
                                    tile_position=(ko, half * 64),
                                )
                        osl = slice(g * 512, (g + 1) * 512)
                        for smp in range(2):
                            dst = st[:, smp, direction, osl]
                            if g < act_evict_banks or (
                                g == 3 and direction == 0
                            ):
                                # ACT fused evict + leaky from PSUM
                                nc.scalar.activation(
                                    out=dst, in_=ps2[smp][:],
                                    func=mybir.ActivationFunctionType.Lrelu,
                                    alpha=alpha01[:],
                                )
                            else:
                                # DVE evict copy PSUM->SBUF bf16 (leaky
                                # applied in the batched stt below)
                                nc.vector.tensor_copy(
                                    out=dst, in_=ps2[smp][:]
                                )
                    # one batched leaky pass over the copy-evicted span
                    # (stt cannot take two PSUM reads); g3/dir0 was
                    # ACT-evicted so that span stops at bank g3
                    dlo = act_evict_banks * 512
                    dhi = 1536 if direction == 0 else 2048
                    for smp in range(2):
                        dvs = st[:, smp, direction, dlo:dhi]
                        nc.vector.scalar_tensor_tensor(
                            out=dvs,
                            in0=dvs,
                            scalar=0.01,
                            in1=dvs,
                            op0=mybir.AluOpType.mult,
                            op1=mybir.AluOpType.max,
                        )
                if grp == ngrp - 1 and j == 1:
                    # fan the tail store across queues so the final
                    # drain is short
                    for smp in range(2):
                        for direction in range(2):
                            nc.sync.dma_start(
                                out=outd[2 * grp + j, :, smp, direction],
                                in_=st[:, smp, direction],
                            )
                else:
                    nc.sync.dma_start(out=outd[2 * grp + j], in_=st[:])

    nc.compile()
    return nc


_NC_CACHE = {}


def _get_nc(bcore=B_CORE):
    if bcore not in _NC_CACHE:
        _NC_CACHE[bcore] = build_nc(bcore)
    return _NC_CACHE[bcore]


def _extract_bands(strips):
    """strips: [bcore//2, 128, 2, 2, 2048] float32-ish -> [bcore, 18, H, W].

    Gram line L (h for hori, w for verti) of sample 2*pr+smp: G_L[r, c] =
      strips[pr, (L%2)*64 + r, smp, dir, (L//16)*512 + ((L%16)//2)*64 + c].
    hori[d, h, w] = G_h[w, w+d-4]; verti[d, h, w] = Gv_w[h, h+d-4].
    """
    bcore = strips.shape[0] * 2
    s = np.asarray(strips, dtype=np.float32)
    # [pr, half(2), r(64), smp(2), dir(2), bank(4), slot(8), c(64)]
    s = s.reshape(bcore // 2, 2, 64, 2, 2, 4, 8, 64)
    # line index L = bank*16 + slot*2 + half -> G[(pr, smp), dir, L, r, c]
    g = s.transpose(0, 3, 4, 5, 6, 1, 2, 7).reshape(bcore, 2, 64, 64, 64)
    out = np.zeros((bcore, 2, ND, 64, 64), dtype=np.float32)
    idx = np.arange(64)
    for d in range(ND):
        o = d - SR
        lo, hi = max(0, -o), min(64, 64 - o)
        r = idx[lo:hi]
        # advanced idxs (incl. the int) are slice-separated -> dims lead:
        # result [len(r), b, L]
        hvals = g[:, 0, :, r, r + o]  # [w-valid, b, h=L]
        vvals = g[:, 1, :, r, r + o]  # [h-valid, b, w=L]
        out[:, 0, d, :, lo:hi] = hvals.transpose(1, 2, 0)
        out[:, 1, d, lo:hi, :] = vvals.transpose(1, 0, 2)
    return out.reshape(bcore, 2 * ND, 64, 64)


def _stage(x):
    """[B, C, H, W] f32 -> bf16 in [B//4, (s%2)*64+c, (s//2)%2, H, W]."""
    b = x.shape[0]
    xb = np.asarray(x, dtype=np.float32).astype(ml_dtypes.bfloat16)
    # sample s = 4*grp + 2*j + half -> [grp, j, half, c, h, w]
    xb = xb.reshape(b // 4, 2, 2, C, H, W)
    # -> [grp, (half, c), j, h, w]
    return np.ascontiguousarray(xb.transpose(0, 2, 3, 1, 4, 5)).reshape(
        b // 4, 128, 2, H, W
    )


def kernel(feature1, feature2, search_range):
    assert int(search_range) == SR
    f1 = _stage(feature1)
    f2 = _stage(feature2)
    bcore = f1.shape[0] * 4 // N_CORES
    nc = _get_nc(bcore)

    from concourse.bass_utils import run_bass_kernel_spmd

    gcore = bcore // 4
    in_maps = [
        {
            "f1": f1[c * gcore : (c + 1) * gcore],
            "f2": f2[c * gcore : (c + 1) * gcore],
        }
        for c in range(N_CORES)
    ]
    res = run_bass_kernel_spmd(nc, in_maps, list(range(N_CORES)))
    outs = [
        _extract_bands(res.results[c]["out"].astype(np.float32))
        for c in range(N_CORES)
    ]
    return np.concatenate(outs, axis=0)


# revision 69
# speedup vs baseline: 1.0439x; 1.0109x over previous
"""Trainium2 Bass kernel for nn_NetworkStitch_5016521802529.

Cost-volume block: f1n = l2_normalize_c(feature1); hori/verti 9-offset
correlation bands vs feature2; leaky_relu; concat -> [B, 18, H, W].

Strategy (pure data-parallel over batch, 32 samples/core on 8 cores):
  - Host stages inputs as bf16 (bit-identical to the on-device cast),
    pre-permuted to the SBUF tile layout; HWDGE loads 4 samples per
    2 MB DMA into [128, 2, H, W] tiles (sample half on partition
    halves, pair slot on dim 1).
  - s^2 = sum_c f1^2 via DVE square + ones-stationary matmul (output
    replicated across partitions); single ACT Abs_reciprocal_sqrt
    (scale=64^2) gives s_inv = 1/(64*s) in the same layout.
  - f1n = f1b * s_inv (bf16); both pairs' norms batched per group so
    the ACT table only switches rsqrt<->lrelu twice per group.
  - Per h (hori) / per w (verti): single-line Gram G = f1n_l^T @ f2b_l
    ([64,64], K=64) on 64x64 PE quadrants; the two samples' matmuls are
    interleaved so consecutive LDWEIGHTS hit different row groups (PE
    pulls them ahead of in-flight MATMULs). 16 Grams per PSUM bank
    [128, 512] (partition half = line%2, free slot = (line%16)//2,
    bank = line//16); each bank is its own pool generation (bufs=6)
    so PSUM recycles per-bank.
  - Eviction PSUM->SBUF fused with leaky-relu: banks g0,g1 (+ g3 on
    hori) via ACT Lrelu (alpha as [128,1] AP -- fewer table loads),
    the rest via DVE copy + one batched scalar_tensor_tensor per
    (sample, direction) -- balances the two eviction-capable engines.
  - Per pair one [128, 2, 2, 2048] bf16 strip (samples x directions)
    stored with a single 2 MB DMA (the last one fanned across queues);
    host extracts the 9-diagonal bands (pure indexing) during unshard.
"""

import os
import sys

sys.path.insert(0, "/opt/trn_rl_repo")

import numpy as np
import ml_dtypes

import concourse.bacc as bacc
import concourse.bass as bass
import concourse.tile as tile
from concourse import mybir

B, C, H, W = 256, 64, 64, 64
N_CORES = 8
B_CORE = B // N_CORES
SR = 4
ND = 2 * SR + 1  # 9
HW = H * W
BF16 = mybir.dt.bfloat16
F32 = mybir.dt.float32

# Eviction split: banks < this evicted by ACT(Lrelu), rest by DVE
# (copy + stt; stt cannot read two PSUM operands).
ACT_EVICT_BANKS = 2


def build_nc(bcore=B_CORE, act_evict_banks=ACT_EVICT_BANKS):
    """Build the per-core Bass graph for `bcore` samples (mult of 4)."""
    assert bcore % 4 == 0
    nc = bacc.Bacc("TRN2", target_bir_lowering=False, debug=False)
    # Inputs staged as bf16 by the host (bit-identical to the bf16 cast
    # the kernel would do on load anyway) -- halves input HBM traffic
    # and lets the loads use HWDGE instead of casting SWDGE. The host
    # also pre-permutes them into the exact SBUF tile layout
    # [group, partition = (s%2)*64 + c, pair slot = (s//2)%2, h, w] so
    # each load is one fully contiguous descriptor pattern.
    f1d = nc.dram_tensor(
        "f1", [bcore // 4, 128, 2, H, W], BF16, kind="ExternalInput"
    )
    f2d = nc.dram_tensor(
        "f2", [bcore // 4, 128, 2, H, W], BF16, kind="ExternalInput"
    )
    # per pair: [128 partitions, 2 samples, 2 directions, 2048] bf16 strips
    outd = nc.dram_tensor(
        "out", [bcore // 2, 128, 2, 2, 4 * 512], BF16, kind="ExternalOutput"
    )
    ngrp = bcore // 4

    from contextlib import ExitStack

    with tile.TileContext(nc) as tc, ExitStack() as ctx:
        io = ctx.enter_context(tc.tile_pool(name="io", bufs=2))
        work = ctx.enter_context(tc.tile_pool(name="work", bufs=2))
        strips = ctx.enter_context(tc.tile_pool(name="strips", bufs=3))
        const = ctx.enter_context(tc.tile_pool(name="const", bufs=1))
        gram = ctx.enter_context(tc.tile_pool(name="gram", bufs=6, space="PSUM"))
        nrm = ctx.enter_context(tc.tile_pool(name="nrm", bufs=2, space="PSUM"))

        ones_t = const.tile([128, 64], BF16)
        nc.vector.memset(ones_t[:], 1.0)
        alpha01 = const.tile([128, 1], F32)
        nc.vector.memset(alpha01[:], 0.01)

        for grp in range(ngrp):
            # ---- loads: 4 samples -> [128, 2, H, W] bf16 ----
            if grp == 0:
                # cold start: per-pair tiles with their own 1 MB loads so
                # pair-0 compute begins as soon as its slice lands (deps
                # are tile-granular)
                f1v, f2v = [], []
                for j in range(2):
                    f1p = io.tile([128, H, W], BF16, name=f"f1p{j}",
                                  tag=f"f1p{j}", bufs=1)
                    f2p = io.tile([128, H, W], BF16, name=f"f2p{j}",
                                  tag=f"f2p{j}", bufs=1)
                    nc.sync.dma_start(out=f1p[:], in_=f1d[0][:, j])
                    nc.sync.dma_start(out=f2p[:], in_=f2d[0][:, j])
                    f1v.append(f1p[:])
                    f2v.append(f2p[:])
            else:
                f1b = io.tile([128, 2, H, W], BF16, tag="f1b")
                f2b = io.tile([128, 2, H, W], BF16, tag="f2b")
                nc.sync.dma_start(out=f1b[:], in_=f1d[grp])
                nc.sync.dma_start(out=f2b[:], in_=f2d[grp])
                f1v = [f1b[:, 0], f1b[:, 1]]
                f2v = [f2b[:, 0], f2b[:, 1]]

            # ---- norms for both pairs first (batches the ACT rsqrt ops
            # so the activation table only switches twice per group) ----
            f1ns = []
            for j in range(2):  # pair slot within the 4-sample group
                f1sq = work.tile([128, HW], BF16, name=f"f1sq{j}",
                                 tag="f1sq")
                nc.vector.tensor_mul(
                    f1sq[:],
                    f1v[j].rearrange("p h w -> p (h w)"),
                    f1v[j].rearrange("p h w -> p (h w)"),
                )
                s_inv = work.tile([128, HW], BF16, name=f"sinv{j}",
                                  tag="sinv")
                for ch in range(8):
                    ps = nrm.tile([128, 512], F32, tag="nps")
                    sl = slice(ch * 512, (ch + 1) * 512)
                    for half in range(2):
                        po = half * 64
                        nc.tensor.matmul(
                            out=ps[po : po + 64, :],
                            lhsT=ones_t[po : po + 64, :],
                            rhs=f1sq[po : po + 64, sl],
                            tile_position=(po, po),
                        )
                    # 1/sqrt(64^2 * ps) = 1/(64*s) in one ACT op (sum >=
                    # 0 so the |x| in Abs_reciprocal_sqrt is a no-op).
                    nc.scalar.activation(
                        out=s_inv[:, sl], in_=ps[:],
                        func=mybir.ActivationFunctionType.Abs_reciprocal_sqrt,
                        scale=float(64 * 64),
                    )

                f1n = work.tile([128, H, W], BF16, name=f"f1n{j}",
                                tag="f1n")
                nc.vector.tensor_mul(
                    f1n[:].rearrange("p h w -> p (h w)"),
                    f1v[j].rearrange("p h w -> p (h w)"),
                    s_inv[:],
                )
                f1ns.append(f1n)

            for j in range(2):
                f1n = f1ns[j]
                # ---- Grams (samples interleaved for LDW overlap) ----
                st = strips.tile([128, 2, 2, 4 * 512], BF16, tag="strip")
                for direction in range(2):  # 0 = hori (per h), 1 = verti
                    for g in range(4):  # bank group of 16 lines
                        # one tag, two calls: each bank is its own pool
                        # generation so PSUM recycles per-bank, not
                        # per-bank-pair
                        ps2 = [
                            gram.tile([128, 512], F32,
                                      name=f"gps{s}", tag="gps")
                            for s in range(2)
                        ]
                        for i in range(16):
                            line = g * 16 + i
                            half, slot = i % 2, i // 2
                            for smp in range(2):
                                ko = smp * 64
                                if direction == 0:
                                    lhsT = f1n[ko : ko + 64, line, :]
                                    rhs = f2v[j][ko : ko + 64, line, :]
                                else:
                                    lhsT = f1n[ko : ko + 64, :, line]
                                    rhs = f2v[j][ko : ko + 64, :, line]
                                nc.tensor.matmul(
                                    out=ps2[smp][
                                        half * 64 : half * 64 + 64,
                                        slot * 64 : slot * 64 + 64,
                                    ],
                                    lhsT=lhsT,
                                    rhs=rhs,
                                    tile_position=(ko, half * 64),
                                )
                        osl = slice(g * 512, (g + 1) * 512)
                        for smp in range(2):
                            dst = st[:, smp, direction, osl]
                            if g < act_evict_banks or (
                                g == 3 and direction == 0
                            ):
                                # ACT fused evict + leaky from PSUM
                                nc.scalar.activation(
                                    out=dst, in_=ps2[smp][:],
                                    func=mybir.ActivationFunctionType.Lrelu,
                                    alpha=alpha01[:],
                                )
                            else:
                                # DVE evict copy PSUM->SBUF bf16 (leaky
                                # applied in the batched stt below)
                                nc.vector.tensor_copy(
                                    out=dst, in_=ps2[smp][:]
                                )
                    # one batched leaky pass over the copy-evicted span
                    # (stt cannot take two PSUM reads); g3/dir0 was
                    # ACT-evicted so that span stops at bank g3
                    dlo = act_evict_banks * 512
                    dhi = 1536 if direction == 0 else 2048
                    for smp in range(2):
                        dvs = st[:, smp, direction, dlo:dhi]
                        nc.vector.scalar_tensor_tensor(
                            out=dvs,
                            in0=dvs,
                            scalar=0.01,
                            in1=dvs,
                            op0=mybir.AluOpType.mult,
                            op1=mybir.AluOpType.max,
                        )
                if grp == ngrp - 1 and j == 1:
                    # fan the tail store across queues so the final
                    # drain is short
                    for smp in range(2):
                        for direction in range(2):
                            nc.sync.dma_start(
                                out=outd[2 * grp + j, :, smp, direction],
                                in_=st[:, smp, direction],
                            )
                else:
                    nc.sync.dma_start(out=outd[2 * grp + j], in_=st[:])

    nc.compile()
    return nc


_NC_CACHE = {}


def _get_nc(bcore=B_CORE):
    if bcore not in _NC_CACHE:
        _NC_CACHE[bcore] = build_nc(bcore)
    return _NC_CACHE[bcore]


def _extract_bands(strips):
    """strips: [bcore//2, 128, 2, 2, 2048] float32-ish -> [bcore, 18, H, W].

    Gram line L (h for hori, w for verti) of sample 2*pr+smp: G_L[r, c] =
      strips[pr, (L%2)*64 + r, smp, dir, (L//16)*512 + ((L%16)//2)*64 + c].
    hori[d, h, w] = G_h[w, w+d-4]; verti[d, h, w] = Gv_w[h, h+d-4].
    """
    bcore = strips.shape[0] * 2
    s = np.asarray(strips, dtype=np.float32)
    # [pr, half(2), r(64), smp(2), dir(2), bank(4), slot(8), c(64)]
    s = s.reshape(bcore // 2, 2, 64, 2, 2, 4, 8, 64)
    # line index L = bank*16 + slot*2 + half -> G[(pr, smp), dir, L, r, c]
    g = s.transpose(0, 3, 4, 5, 6, 1, 2, 7).reshape(bcore, 2, 64, 64, 64)
    out = np.zeros((bcore, 2, ND, 64, 64), dtype=np.float32)
    idx = np.arange(64)
    for d in range(ND):
        o = d - SR
        lo, hi = max(0, -o), min(64, 64 - o)
        r = idx[lo:hi]
        # advanced idxs (incl. the int) are slice-separated -> dims lead:
        # result [len(r), b, L]
        hvals = g[:, 0, :, r, r + o]  # [w-valid, b, h=L]
        vvals = g[:, 1, :, r, r + o]  # [h-valid, b, w=L]
        out[:, 0, d, :, lo:hi] = hvals.transpose(1, 2, 0)
        out[:, 1, d, lo:hi, :] = vvals.transpose(1, 0, 2)
    return out.reshape(bcore, 2 * ND, 64, 64)


def _stage(x):
    """[B, C, H, W] f32 -> bf16 in [B//4, (s%2)*64+c, (s//2)%2, H, W]."""
    b = x.shape[0]
    xb = np.asarray(x, dtype=np.float32).astype(ml_dtypes.bfloat16)
    # sample s = 4*grp + 2*j + half -> [grp, j, half, c, h, w]
    xb = xb.reshape(b // 4, 2, 2, C, H, W)
    # -> [grp, (half, c), j, h, w]
    return np.ascontiguousarray(xb.transpose(0, 2, 3, 1, 4, 5)).reshape(
        b // 4, 128, 2, H, W
    )


def kernel(feature1, feature2, search_range):
    assert int(search_range) == SR
    f1 = _stage(feature1)
    f2 = _stage(feature2)
    bcore = f1.shape[0] * 4 // N_CORES
    nc = _get_nc(bcore)

    from concourse.bass_utils import run_bass_kernel_spmd

    gcore = bcore // 4
    in_maps = [
        {
            "f1": f1[c * gcore : (c + 1) * gcore],
            "f2": f2[c * gcore : (c + 1) * gcore],
        }
        for c in range(N_CORES)
    ]
    res = run_bass_kernel_spmd(nc, in_maps, list(range(N_CORES)))
    outs = [
        _extract_bands(res.results[c]["out"].astype(np.float32))
        for c in range(N_CORES)
    ]
    return np.concatenate(outs, axis=0)
